# revision 2
# baseline (speedup 1.0000x reference)
"""Trainium2 Bass kernel for nn_AttentionTemporelle (3-window banded attention).

v2: transposed-scores design, bf16 datapath, DMA-XBAR transposes.

Per batch element (data-parallel over B=8, one per core):
    q = x @ Wq ; k = x @ Wk                     [T, DK]
    s = q k^T / sqrt(DK); 3 banded softmaxes averaged; @x; @Wo; +x; LayerNorm

Key structure (vs v1):
  * Scores computed TRANSPOSED per 128-row block i: for each j-block in the
    +-3-block strip, sT[j,t] = matmul(lhsT=kT_j, rhs=qT_i) -> psum [128, nb*128].
    exp() writes em[j,t] straight to SBUF bf16 -- the PV matmul consumes em as
    lhsT directly, so v1's 13 PE transposes + psum->sbuf copy per block vanish.
  * Window merge: em_mid += e168*c168(t) + e24*c24(t), where the per-t scales
    c168 = Z720/Z168 etc. Z's come from N=1 matmuls (lhsT=em tiles, rhs=ones),
    the scale row-vector from one tiny PE transpose + gpsimd partition
    broadcasts. One PV (7 matmuls vs x@Wo/3) then computes all three windows.
  * rstd = exp(-0.5*ln(var+eps)): keeps every ACT func in one act table
    (natural_log_exp_and_others) -> a single LoadActFuncSet for the kernel.
  * bf16 everywhere on the PE/DVE path (fp32 psum accumulation), bf16 HBM I/O,
    xT via DMA XBAR transpose (16 chunk transposes, no PE work).
  * LayerNorm folded per quarter; output DMA'd per quarter (bf16).
  * PE emission is software-pipelined: step i emits sT(i), PV(i-2) part A,
    Z(i-1), PV(i-2) part B, ccT(i-1) so cross-engine chains have ~2 block
    periods of slack and the PE queue never drains.
"""

import math

import numpy as np

B, T, D, DK = 8, 2048, 512, 128
NBLK = T // 128                 # 16 row blocks
HALO = 3                        # 360 // 128 + 1 neighbor blocks each side
STRIP = (2 * HALO + 1) * 128    # 896
EPS = 1e-5
H720, H168, H24 = 360, 84, 12

_CACHE = {}


def _host_consts():
    import ml_dtypes

    bf = ml_dtypes.bfloat16
    p = np.arange(128)[:, None, None]
    jb7 = np.arange(7)[None, :, None]
    tt = np.arange(128)[None, None, :]
    d7 = (jb7 - HALO) * 128 + p - tt          # j_global - t, canonical strip
    band01T = (np.abs(d7) <= H720).astype(bf).reshape(128, STRIP)
    jb3 = np.arange(3)[None, :, None]
    d3 = (jb3 - 1) * 128 + p - tt
    m168T = (np.abs(d3) <= H168).astype(bf).reshape(128, 384)
    m24T = (np.abs(d3) <= H24).astype(bf).reshape(128, 384)
    ident = np.eye(128, dtype=np.float32).astype(bf)
    return band01T, m168T, m24T, ident


def _build_nc(has_bq, has_bk, has_bo, has_gamma, has_beta):
    import concourse.bass as bass
    import concourse.tile as tile
    from concourse import bacc, mybir

    f32 = mybir.dt.float32
    bf16 = mybir.dt.bfloat16
    AF = mybir.ActivationFunctionType
    OP = mybir.AluOpType

    nc = bacc.Bacc()

    x_d = nc.declare_dram_parameter("x", [T, D], bf16, isOutput=False)
    wqk_d = nc.declare_dram_parameter("Wqk_s", [D, 2 * DK], bf16, isOutput=False)
    wo_d = nc.declare_dram_parameter("Wo", [D, D], bf16, isOutput=False)
    # band01T | m168T | m24T | ident packed along the free dim (one DMA)
    csts_d = nc.declare_dram_parameter(
        "csts", [128, STRIP + 384 + 384 + 128], bf16, isOutput=False
    )
    if has_bq:
        bq_d = nc.declare_dram_parameter("bq_s", [DK, 1], f32, isOutput=False)
    if has_bk:
        bk_d = nc.declare_dram_parameter("bk_c", [DK, 1], f32, isOutput=False)
    if has_bo:
        ones_d = nc.declare_dram_parameter("ones_row", [1, 128], bf16, isOutput=False)
        bo_d = nc.declare_dram_parameter("bo_row", [1, D], bf16, isOutput=False)
    if has_gamma:
        gamma_d = nc.declare_dram_parameter("gamma_bc", [128, D], f32, isOutput=False)
    if has_beta:
        beta_d = nc.declare_dram_parameter("beta_bc", [128, D], f32, isOutput=False)
    out_d = nc.declare_dram_parameter("out", [T, D], bf16, isOutput=True)

    with tile.TileContext(nc) as tc:
        with tc.tile_pool(name="persist", bufs=1) as persist:
            x_tiles = [
                persist.tile([128, 4, D], bf16, tag=f"x{g}", name=f"x_sb{g}")
                for g in range(4)
            ]
            # per-(chunk, quarter) tiles: a single [128, T] tile per chunk
            # false-shares across quarters (q1's transpose DMA would wait on
            # every q0 reader), so keep them separate.
            xT_cq = [
                [
                    persist.tile(
                        [128, 512], bf16, tag=f"xT{c}_{q}", name=f"xT_sb{c}_{q}"
                    )
                    for q in range(4)
                ]
                for c in range(4)
            ]
            qT_q = [
                persist.tile([128, 512], bf16, tag=f"qT{g}", name=f"qT_sb{g}")
                for g in range(4)
            ]
            kT_q = [
                persist.tile([128, 512], bf16, tag=f"kT{g}", name=f"kT_sb{g}")
                for g in range(4)
            ]
            xWo_q = [
                persist.tile([128, 4, D], bf16, tag=f"xWo{g}", name=f"xWo_sb{g}")
                for g in range(4)
            ]
            wqk_sb = persist.tile([128, 4, 2 * DK], bf16, tag="wqk")
            wq_sb = wqk_sb[:, :, 0:DK]
            wk_sb = wqk_sb[:, :, DK:2 * DK]
            wo_sb = persist.tile([128, 4, D], bf16, tag="wo")
            csts_sb = persist.tile(
                [128, STRIP + 384 + 384 + 128], bf16, tag="csts"
            )
            band_sb = csts_sb[:, 0:STRIP]
            m168_sb = csts_sb[:, STRIP:STRIP + 384]
            m24_sb = csts_sb[:, STRIP + 384:STRIP + 768]
            ident_sb = csts_sb[:, STRIP + 768:STRIP + 896]
            ones_sb = persist.tile([128, 1], bf16, tag="ones")
            nc.vector.memset(ones_sb, 1.0)
            eps_sb = persist.tile([128, 1], f32, tag="eps")
            nc.vector.memset(eps_sb, EPS)
            res16 = persist.tile([128, NBLK, D], f32, tag="res16")
            rsum16 = persist.tile([128, NBLK], f32, tag="rsum16")
            sqsum16 = persist.tile([128, NBLK], f32, tag="sqsum16")

            x_r = x_d[:].rearrange("(n p) d -> p n d", p=128)

            def xt_dma(q):
                for c in range(4):
                    nc.sync.dma_start(
                        out=xT_cq[c][q],
                        in_=x_d[:][q * 512:(q + 1) * 512, c * 128:(c + 1) * 128],
                        transpose=True,
                    )

            # critical-path order: wqk + xT(q0) gate the first projections,
            # wo gates xWo(0), masks/ident gate the first p1 blocks, x0 the
            # first residual.
            nc.sync.dma_start(
                out=wqk_sb, in_=wqk_d[:].rearrange("(c p) k -> p c k", p=128)
            )
            xt_dma(0)
            nc.sync.dma_start(
                out=wo_sb, in_=wo_d[:].rearrange("(c p) k -> p c k", p=128)
            )
            xt_dma(1)
            nc.sync.dma_start(out=csts_sb, in_=csts_d[:])
            nc.sync.dma_start(out=x_tiles[0], in_=x_r[:, 0:4, :])
            xt_dma(2)
            nc.sync.dma_start(out=x_tiles[1], in_=x_r[:, 4:8, :])
            xt_dma(3)
            nc.sync.dma_start(out=x_tiles[2], in_=x_r[:, 8:12, :])
            nc.sync.dma_start(out=x_tiles[3], in_=x_r[:, 12:16, :])
            if has_bq:
                bq_sb = persist.tile([128, 1], f32, tag="bq")
                nc.sync.dma_start(out=bq_sb, in_=bq_d[:])
            if has_bk:
                bk_sb = persist.tile([128, 1], f32, tag="bk")
                nc.sync.dma_start(out=bk_sb, in_=bk_d[:])
            if has_bo:
                ones1p_sb = persist.tile([1, 128], bf16, tag="ones1p")
                bo_sb = persist.tile([1, D], bf16, tag="bo")
                nc.sync.dma_start(out=ones1p_sb, in_=ones_d[:])
                nc.sync.dma_start(out=bo_sb, in_=bo_d[:])
            if has_gamma:
                gamma_sb = persist.tile([128, D], f32, tag="gamma")
                nc.sync.dma_start(out=gamma_sb, in_=gamma_d[:])
            if has_beta:
                beta_sb = persist.tile([128, D], f32, tag="beta")
                nc.sync.dma_start(out=beta_sb, in_=beta_d[:])

            with (
                tc.tile_pool(name="ps0", bufs=2, space="PSUM") as ps0,
                tc.tile_pool(name="s_ps", bufs=1, space="PSUM") as s_ps,
                tc.tile_pool(name="z_ps", bufs=1, space="PSUM") as z_ps,
                tc.tile_pool(name="ct_ps", bufs=1, space="PSUM") as ct_ps,
                tc.tile_pool(name="acc_ps", bufs=2, space="PSUM") as acc_ps,
                tc.tile_pool(name="work", bufs=2) as work,
                tc.tile_pool(name="small", bufs=3) as small,
            ):
                def p0_quarter(tq):
                    # qT / kT for this quarter
                    for idx, (w_sb, dst_q, bias_sb) in enumerate((
                        (wq_sb, qT_q, bq_sb if has_bq else None),
                        (wk_sb, kT_q, bk_sb if has_bk else None),
                    )):
                        pr = ps0.tile([128, 512], f32, tag="ps0", name="pr_ps")
                        for c in range(4):
                            nc.tensor.matmul(
                                out=pr,
                                lhsT=w_sb[:, c, :],
                                rhs=xT_cq[c][tq],
                                start=(c == 0),
                                stop=(c == 3),
                            )
                        if bias_sb is not None:
                            nc.scalar.activation(
                                out=dst_q[tq][:, :],
                                in_=pr,
                                func=AF.Identity,
                                bias=bias_sb[:, :],
                                scale=1.0,
                            )
                        else:
                            nc.scalar.activation(
                                out=dst_q[tq][:, :], in_=pr, func=AF.Copy
                            )

                def emit_xwo(ti):
                    # xWo for one row block (interleaved into pipeline steps
                    # so the PE has phase-0 filler during the p1 drain)
                    tq, tl = ti // 4, ti % 4
                    xw = ps0.tile([128, 512], f32, tag="ps0", name="xw_ps")
                    for c in range(4):
                        nc.tensor.matmul(
                            out=xw,
                            lhsT=xT_cq[c][tq][:, tl * 128:(tl + 1) * 128],
                            rhs=wo_sb[:, c, :],
                            start=(c == 0),
                            stop=(c == 3 and not has_bo),
                        )
                    if has_bo:
                        nc.tensor.matmul(
                            out=xw,
                            lhsT=ones1p_sb[:, :],
                            rhs=bo_sb[:, :],
                            start=False,
                            stop=True,
                        )
                    if ti % 4 == 0:
                        nc.vector.tensor_copy(out=xWo_q[tq][:, tl, :], in_=xw)
                    else:
                        nc.scalar.activation(
                            out=xWo_q[tq][:, tl, :], in_=xw, func=AF.Copy
                        )

                # per-block state for the software pipeline
                st = {}

                def geom(i):
                    jlo, jhi = max(0, i - HALO), min(NBLK - 1, i + HALO)
                    nb = jhi - jlo + 1
                    cb0 = jlo - i + HALO
                    mlo, mhi = max(0, i - 1), min(NBLK - 1, i + 1)
                    return jlo, jhi, nb, cb0, mlo, mhi

                def emit_sT_exp_masks(i):
                    jlo, jhi, nb, cb0, mlo, mhi = geom(i)
                    ncols = nb * 128
                    s_t = s_ps.tile([128, STRIP], f32, tag="s")
                    for p_ in range(nb):
                        j = jlo + p_
                        nc.tensor.matmul(
                            out=s_t[:, p_ * 128:(p_ + 1) * 128],
                            lhsT=kT_q[j // 4][:, (j % 4) * 128:(j % 4 + 1) * 128],
                            rhs=qT_q[i // 4][:, (i % 4) * 128:(i % 4 + 1) * 128],
                            start=True,
                            stop=True,
                        )
                    em = work.tile([128, STRIP], bf16, tag="em", bufs=5)
                    nc.scalar.activation(
                        out=em[:, :ncols], in_=s_t[:, :ncols], func=AF.Exp
                    )
                    # band mask: out-of-band parts of blocks |j-i| >= 2
                    lo_cols = (max(0, i - 1) - jlo) * 128
                    hi_cols = (min(NBLK - 1, i + 1) - jlo + 1) * 128
                    co = cb0 * 128
                    if lo_cols > 0:
                        nc.vector.tensor_mul(
                            out=em[:, :lo_cols],
                            in0=em[:, :lo_cols],
                            in1=band_sb[:, co:co + lo_cols],
                        )
                    if hi_cols < ncols:
                        nc.vector.tensor_mul(
                            out=em[:, hi_cols:ncols],
                            in0=em[:, hi_cols:ncols],
                            in1=band_sb[:, co + hi_cols:co + ncols],
                        )
                    # inner-window masked copies
                    ms = (mlo - jlo) * 128
                    mc = (mlo - i + 1) * 128
                    mcols = (mhi - mlo + 1) * 128
                    e168 = work.tile([128, 384], bf16, tag="e168", bufs=3)
                    e24 = work.tile([128, 384], bf16, tag="e24", bufs=3)
                    nc.gpsimd.tensor_mul(
                        out=e168[:, :mcols],
                        in0=em[:, ms:ms + mcols],
                        in1=m168_sb[:, mc:mc + mcols],
                    )
                    nc.gpsimd.tensor_mul(
                        out=e24[:, :mcols],
                        in0=em[:, ms:ms + mcols],
                        in1=m24_sb[:, mc:mc + mcols],
                    )
                    st[i] = dict(em=em, e168=e168, e24=e24)

                def emit_z(i):
                    jlo, jhi, nb, cb0, mlo, mhi = geom(i)
                    d = st[i]
                    z3 = z_ps.tile([128, 4], f32, tag="z3")
                    mcols = (mhi - mlo + 1) * 128
                    nm = mcols // 128
                    for p_ in range(nb):
                        nc.tensor.matmul(
                            out=z3[:, 0:1],
                            lhsT=d["em"][:, p_ * 128:(p_ + 1) * 128],
                            rhs=ones_sb,
                            start=(p_ == 0),
                            stop=(p_ == nb - 1),
                        )
                    for col, tname in ((1, "e168"), (2, "e24")):
                        for m_ in range(nm):
                            nc.tensor.matmul(
                                out=z3[:, col:col + 1],
                                lhsT=d[tname][:, m_ * 128:(m_ + 1) * 128],
                                rhs=ones_sb,
                                start=(m_ == 0),
                                stop=(m_ == nm - 1),
                            )
                    rcp = small.tile([128, 3], f32, tag="rcp", bufs=4)
                    nc.vector.reciprocal(out=rcp, in_=z3[:, 0:3])
                    z3s = small.tile([128, 1], f32, tag="z3s", bufs=2)
                    nc.vector.tensor_copy(out=z3s, in_=z3[:, 0:1])
                    cc = small.tile([128, 2], bf16, tag="cc", bufs=2)
                    nc.vector.tensor_scalar(
                        out=cc,
                        in0=rcp[:, 1:3],
                        scalar1=z3s[:, 0:1],
                        scalar2=None,
                        op0=OP.mult,
                    )
                    d["rcp"] = rcp
                    d["cc"] = cc

                def emit_ccchain(i):
                    # ccT transpose -> ccrow -> partition broadcast -> scale &
                    # merge inner windows into em (runs well before PV(i)).
                    jlo, jhi, nb, cb0, mlo, mhi = geom(i)
                    d = st[i]
                    # two single-row transposes: every consumer AP must
                    # start at partition 0 (BIR verifier rejects base=1)
                    ct = ct_ps.tile([1, 256], bf16, tag="ct", name="ct")
                    nc.tensor.matmul(
                        out=ct[:, 0:128],
                        lhsT=d["cc"][:, 0:1],
                        rhs=ident_sb,
                        is_transpose=True,
                        start=True,
                        stop=True,
                    )
                    nc.tensor.matmul(
                        out=ct[:, 128:256],
                        lhsT=d["cc"][:, 1:2],
                        rhs=ident_sb,
                        is_transpose=True,
                        start=True,
                        stop=True,
                    )
                    ccrow = small.tile([1, 256], bf16, tag="ccrow", bufs=2)
                    nc.vector.tensor_copy(out=ccrow, in_=ct)
                    ccb = small.tile([128, 256], bf16, tag="ccb", bufs=2)
                    nc.gpsimd.partition_broadcast(ccb[:, 0:128], ccrow[:, 0:128])
                    nc.gpsimd.partition_broadcast(ccb[:, 128:256], ccrow[:, 128:256])
                    ms = (mlo - jlo) * 128
                    mcols = (mhi - mlo + 1) * 128
                    nm = mcols // 128
                    em, e168, e24 = d["em"], d["e168"], d["e24"]
                    for tname, coff in (("e168", 0), ("e24", 128)):
                        tl = d[tname]
                        t3 = tl[:, :mcols].rearrange("p (m t) -> p m t", m=nm)
                        cb = ccb[:, coff:coff + 128]
                        cb3 = bass.AP(
                            tensor=cb.tensor,
                            offset=cb.offset,
                            ap=[cb.ap[0], [0, nm], [1, 128]],
                        )
                        nc.vector.tensor_mul(out=t3, in0=t3, in1=cb3)
                    nc.vector.tensor_add(
                        out=em[:, ms:ms + mcols],
                        in0=em[:, ms:ms + mcols],
                        in1=e168[:, :mcols],
                    )
                    nc.vector.tensor_add(
                        out=em[:, ms:ms + mcols],
                        in0=em[:, ms:ms + mcols],
                        in1=e24[:, :mcols],
                    )

                def emit_pv(i, part):
                    jlo, jhi, nb, cb0, mlo, mhi = geom(i)
                    d = st[i]
                    if part == 0:
                        d["acc"] = acc_ps.tile([128, 512], f32, tag="acc", name="acc_ps")
                        rng = range(0, min(6, nb))
                    else:
                        rng = range(min(6, nb), nb)
                    for p_ in rng:
                        j = jlo + p_
                        nc.tensor.matmul(
                            out=d["acc"],
                            lhsT=d["em"][:, p_ * 128:(p_ + 1) * 128],
                            rhs=xWo_q[j // 4][:, j % 4, :],
                            start=(p_ == 0),
                            stop=(p_ == nb - 1),
                        )

                def emit_res(i):
                    d = st[i]
                    # res = acc/z720 + x ; row-sum for LN mean (on gpsimd)
                    nc.vector.scalar_tensor_tensor(
                        out=res16[:, i, :],
                        in0=d["acc"],
                        scalar=d["rcp"][:, 0:1],
                        in1=x_tiles[i // 4][:, i % 4, :],
                        op0=OP.mult,
                        op1=OP.add,
                        accum_out=rsum16[:, i:i + 1],
                    )
                    sqscr = work.tile([128, D], f32, tag="sqscr", bufs=2)
                    nc.scalar.activation(
                        out=sqscr,
                        in_=res16[:, i, :],
                        func=AF.Square,
                        accum_out=sqsum16[:, i:i + 1],
                    )
                    del st[i]["acc"]

                def ln_quarter(q, h0=None, hn=4):
                    if h0 is None:
                        h0 = 4 * q
                    hsl = slice(h0, h0 + hn)
                    mu = small.tile([128, hn], f32, tag="mu", bufs=3)
                    nc.vector.tensor_scalar_mul(
                        out=mu, in0=rsum16[:, hsl], scalar1=1.0 / D
                    )
                    musq = small.tile([128, hn], f32, tag="musq", bufs=3)
                    nc.vector.tensor_mul(out=musq, in0=mu, in1=mu)
                    var = small.tile([128, hn], f32, tag="var", bufs=3)
                    nc.vector.tensor_scalar(
                        out=var,
                        in0=sqsum16[:, hsl],
                        scalar1=1.0 / D,
                        scalar2=EPS,
                        op0=OP.mult,
                        op1=OP.add,
                    )
                    nc.vector.tensor_sub(out=var, in0=var, in1=musq)
                    # rstd = 1/sqrt(var) via DVE-only Newton iteration (keeps
                    # the ACT table on exp for the whole kernel): seed from the
                    # hardware reciprocal, y0 = (1+r)/2 ~ sqrt(r), then two
                    # steps of y <- y*(1.5 - 0.5*var*y^2).
                    rv = small.tile([128, hn], f32, tag="rv", bufs=3)
                    nc.vector.reciprocal(out=rv, in_=var)
                    rstd = small.tile([128, hn], f32, tag="rstd", bufs=3)
                    nc.vector.tensor_scalar(
                        out=rstd,
                        in0=rv,
                        scalar1=0.5,
                        scalar2=0.5,
                        op0=OP.mult,
                        op1=OP.add,
                    )
                    u = small.tile([128, hn], f32, tag="u", bufs=3)
                    for _ in range(2):
                        nc.vector.tensor_mul(out=u, in0=rstd, in1=rstd)
                        nc.vector.tensor_mul(out=u, in0=u, in1=var)
                        nc.vector.tensor_scalar(
                            out=u,
                            in0=u,
                            scalar1=-0.5,
                            scalar2=1.5,
                            op0=OP.mult,
                            op1=OP.add,
                        )
                        nc.vector.tensor_mul(out=rstd, in0=rstd, in1=u)
                    nmb = small.tile([128, hn], f32, tag="nmb", bufs=3)
                    # nmb = -mu * rstd (elementwise across the 4 block columns)
                    nc.vector.scalar_tensor_tensor(
                        out=nmb,
                        in0=mu,
                        scalar=-1.0,
                        in1=rstd,
                        op0=OP.mult,
                        op1=OP.mult,
                    )
                    outq = work.tile([128, hn, D], bf16, tag="outq", bufs=3)
                    for k in range(hn):
                        ib = h0 + k
                        if k == 1:
                            nc.scalar.activation(
                                out=outq[:, k, :],
                                in_=res16[:, ib, :],
                                func=AF.Identity,
                                bias=nmb[:, k:k + 1],
                                scale=rstd[:, k:k + 1],
                            )
                        elif k == 3:
                            nc.scalar.activation(
                                out=outq[:, k, :],
                                in_=res16[:, ib, :],
                                func=AF.Identity,
                                bias=nmb[:, k:k + 1],
                                scale=rstd[:, k:k + 1],
                            )
                        else:
                            nc.vector.tensor_scalar(
                                out=outq[:, k, :],
                                in0=res16[:, ib, :],
                                scalar1=rstd[:, k:k + 1],
                                scalar2=nmb[:, k:k + 1],
                                op0=OP.mult,
                                op1=OP.add,
                            )
                        if has_gamma:
                            nc.gpsimd.tensor_mul(
                                out=outq[:, k, :], in0=outq[:, k, :], in1=gamma_sb
                            )
                        if has_beta:
                            nc.gpsimd.tensor_add(
                                out=outq[:, k, :], in0=outq[:, k, :], in1=beta_sb
                            )
                    out_r = out_d[:].rearrange("(n p) d -> p n d", p=128)
                    nc.sync.dma_start(
                        out=out_r[:, h0:h0 + hn, :], in_=outq
                    )

                LAG = 3

                def pipeline_step(i):
                    """Emit work for pipeline step i (i in 0..NBLK+LAG-1)."""
                    if i + HALO < NBLK:
                        emit_xwo(i + HALO)
                    if i < NBLK:
                        emit_sT_exp_masks(i)
                    if i - LAG >= 0:
                        emit_pv(i - LAG, 0)
                    if 0 <= i - 1 < NBLK:
                        emit_z(i - 1)
                    if i - LAG >= 0:
                        emit_pv(i - LAG, 1)
                        emit_res(i - LAG)
                    if 0 <= i - 1 < NBLK:
                        emit_ccchain(i - 1)
                    if i - LAG >= 0 and (i - LAG) % 2 == 1:
                        ln_quarter(None, h0=i - LAG - 1, hn=2)

                done = 0
                for tq in range(4):
                    p0_quarter(tq)
                    if tq == 0:
                        for ti in range(HALO):
                            emit_xwo(ti)
                    while done < NBLK and (min(done + HALO, NBLK - 1)) // 4 <= tq:
                        pipeline_step(done)
                        done += 1
                while done < NBLK + LAG:
                    pipeline_step(done)
                    done += 1

    nc.compile()
    return nc


def _get_built(flags):
    if flags not in _CACHE:
        _CACHE[flags] = _build_nc(*flags)
    return _CACHE[flags]


def _make_in_maps(x, Wq, bq, Wk, bk, Wo, bo, gamma, beta, flags):
    import ml_dtypes

    bf = ml_dtypes.bfloat16
    has_bq, has_bk, has_bo, has_gamma, has_beta = flags
    band01T, m168T, m24T, ident = _host_consts()
    scale = 1.0 / math.sqrt(DK)
    wqk = np.concatenate([(Wq * scale).astype(bf), Wk.astype(bf)], axis=1)
    csts = np.concatenate([band01T, m168T, m24T, ident], axis=1)
    base = {
        "Wqk_s": np.ascontiguousarray(wqk),
        "Wo": np.ascontiguousarray((Wo / 3.0).astype(bf)),
        "csts": np.ascontiguousarray(csts),
    }
    if has_bq:
        base["bq_s"] = np.ascontiguousarray(bq * scale, dtype=np.float32).reshape(DK, 1)
    if has_bk:
        base["bk_c"] = np.ascontiguousarray(bk, dtype=np.float32).reshape(DK, 1)
    if has_bo:
        base["ones_row"] = np.ones((1, 128), dtype=np.float32).astype(bf)
        base["bo_row"] = (bo / 3.0).astype(bf).reshape(1, D)
    if has_gamma:
        base["gamma_bc"] = np.broadcast_to(
            np.asarray(gamma, dtype=np.float32), (128, D)
        ).copy()
    if has_beta:
        base["beta_bc"] = np.broadcast_to(
            np.asarray(beta, dtype=np.float32), (128, D)
        ).copy()
    xb = np.ascontiguousarray(x).astype(bf)
    return [{**base, "x": xb[core]} for core in range(B)]


def kernel(x, Wq, bq, Wk, bk, Wo, bo, gamma, beta):
    from concourse.bass_utils import run_bass_kernel_spmd

    x = np.asarray(x, dtype=np.float32)
    Wq = np.asarray(Wq, dtype=np.float32)
    bq = np.asarray(bq, dtype=np.float32)
    Wk = np.asarray(Wk, dtype=np.float32)
    bk = np.asarray(bk, dtype=np.float32)
    Wo = np.asarray(Wo, dtype=np.float32)
    bo = np.asarray(bo, dtype=np.float32)
    gamma = np.asarray(gamma, dtype=np.float32)
    beta = np.asarray(beta, dtype=np.float32)

    flags = (
        bool(np.any(bq != 0.0)),
        bool(np.any(bk != 0.0)),
        bool(np.any(bo != 0.0)),
        bool(np.any(gamma != 1.0)),
        bool(np.any(beta != 0.0)),
    )
    nc = _get_built(flags)
    in_maps = _make_in_maps(x, Wq, bq, Wk, bk, Wo, bo, gamma, beta, flags)
    res = run_bass_kernel_spmd(nc, in_maps, list(range(B)))
    return np.stack(
        [np.asarray(res.results[c]["out"], dtype=np.float32) for c in range(B)], axis=0
    )


# revision 3
# speedup vs baseline: 1.0008x; 1.0008x over previous
"""Trainium2 Bass kernel for nn_AttentionTemporelle (3-window banded attention).

v2: transposed-scores design, bf16 datapath, DMA-XBAR transposes.

Per batch element (data-parallel over B=8, one per core):
    q = x @ Wq ; k = x @ Wk                     [T, DK]
    s = q k^T / sqrt(DK); 3 banded softmaxes averaged; @x; @Wo; +x; LayerNorm

Key structure (vs v1):
  * Scores computed TRANSPOSED per 128-row block i: for each j-block in the
    +-3-block strip, sT[j,t] = matmul(lhsT=kT_j, rhs=qT_i) -> psum [128, nb*128].
    exp() writes em[j,t] straight to SBUF bf16 -- the PV matmul consumes em as
    lhsT directly, so v1's 13 PE transposes + psum->sbuf copy per block vanish.
  * Window merge: em_mid += e168*c168(t) + e24*c24(t), where the per-t scales
    c168 = Z720/Z168 etc. Z's come from N=1 matmuls (lhsT=em tiles, rhs=ones),
    the scale row-vector from one tiny PE transpose + gpsimd partition
    broadcasts. One PV (7 matmuls vs x@Wo/3) then computes all three windows.
  * rstd = exp(-0.5*ln(var+eps)): keeps every ACT func in one act table
    (natural_log_exp_and_others) -> a single LoadActFuncSet for the kernel.
  * bf16 everywhere on the PE/DVE path (fp32 psum accumulation), bf16 HBM I/O,
    xT via DMA XBAR transpose (16 chunk transposes, no PE work).
  * LayerNorm folded per quarter; output DMA'd per quarter (bf16).
  * PE emission is software-pipelined: step i emits sT(i), PV(i-2) part A,
    Z(i-1), PV(i-2) part B, ccT(i-1) so cross-engine chains have ~2 block
    periods of slack and the PE queue never drains.
"""

import math

import numpy as np

B, T, D, DK = 8, 2048, 512, 128
NBLK = T // 128                 # 16 row blocks
HALO = 3                        # 360 // 128 + 1 neighbor blocks each side
STRIP = (2 * HALO + 1) * 128    # 896
EPS = 1e-5
H720, H168, H24 = 360, 84, 12

_CACHE = {}


def _host_consts():
    import ml_dtypes

    bf = ml_dtypes.bfloat16
    p = np.arange(128)[:, None, None]
    jb7 = np.arange(7)[None, :, None]
    tt = np.arange(128)[None, None, :]
    d7 = (jb7 - HALO) * 128 + p - tt          # j_global - t, canonical strip
    band01T = (np.abs(d7) <= H720).astype(bf).reshape(128, STRIP)
    jb3 = np.arange(3)[None, :, None]
    d3 = (jb3 - 1) * 128 + p - tt
    m168T = (np.abs(d3) <= H168).astype(bf).reshape(128, 384)
    m24T = (np.abs(d3) <= H24).astype(bf).reshape(128, 384)
    ident = np.eye(128, dtype=np.float32).astype(bf)
    return band01T, m168T, m24T, ident


def _build_nc(has_bq, has_bk, has_bo, has_gamma, has_beta):
    import concourse.bass as bass
    import concourse.tile as tile
    from concourse import bacc, mybir

    f32 = mybir.dt.float32
    bf16 = mybir.dt.bfloat16
    AF = mybir.ActivationFunctionType
    OP = mybir.AluOpType

    nc = bacc.Bacc()

    x_d = nc.declare_dram_parameter("x", [T, D], bf16, isOutput=False)
    wqk_d = nc.declare_dram_parameter("Wqk_s", [D, 2 * DK], bf16, isOutput=False)
    wo_d = nc.declare_dram_parameter("Wo", [D, D], bf16, isOutput=False)
    # band01T | m168T | m24T | ident packed along the free dim (one DMA)
    csts_d = nc.declare_dram_parameter(
        "csts", [128, STRIP + 384 + 384 + 128], bf16, isOutput=False
    )
    if has_bq:
        bq_d = nc.declare_dram_parameter("bq_s", [DK, 1], f32, isOutput=False)
    if has_bk:
        bk_d = nc.declare_dram_parameter("bk_c", [DK, 1], f32, isOutput=False)
    if has_bo:
        ones_d = nc.declare_dram_parameter("ones_row", [1, 128], bf16, isOutput=False)
        bo_d = nc.declare_dram_parameter("bo_row", [1, D], bf16, isOutput=False)
    if has_gamma:
        gamma_d = nc.declare_dram_parameter("gamma_bc", [128, D], f32, isOutput=False)
    if has_beta:
        beta_d = nc.declare_dram_parameter("beta_bc", [128, D], f32, isOutput=False)
    out_d = nc.declare_dram_parameter("out", [T, D], bf16, isOutput=True)

    with tile.TileContext(nc) as tc:
        with tc.tile_pool(name="persist", bufs=1) as persist:
            x_tiles = [
                persist.tile([128, 4, D], bf16, tag=f"x{g}", name=f"x_sb{g}")
                for g in range(4)
            ]
            # per-(chunk, quarter) tiles: a single [128, T] tile per chunk
            # false-shares across quarters (q1's transpose DMA would wait on
            # every q0 reader), so keep them separate.
            xT_cq = [
                [
                    persist.tile(
                        [128, 512], bf16, tag=f"xT{c}_{q}", name=f"xT_sb{c}_{q}"
                    )
                    for q in range(4)
                ]
                for c in range(4)
            ]
            qT_q = [
                persist.tile([128, 512], bf16, tag=f"qT{g}", name=f"qT_sb{g}")
                for g in range(4)
            ]
            kT_q = [
                persist.tile([128, 512], bf16, tag=f"kT{g}", name=f"kT_sb{g}")
                for g in range(4)
            ]
            xWo_q = [
                persist.tile([128, 4, D], bf16, tag=f"xWo{g}", name=f"xWo_sb{g}")
                for g in range(4)
            ]
            wqk_sb = persist.tile([128, 4, 2 * DK], bf16, tag="wqk")
            wq_sb = wqk_sb[:, :, 0:DK]
            wk_sb = wqk_sb[:, :, DK:2 * DK]
            wo_sb = persist.tile([128, 4, D], bf16, tag="wo")
            csts_sb = persist.tile(
                [128, STRIP + 384 + 384 + 128], bf16, tag="csts"
            )
            band_sb = csts_sb[:, 0:STRIP]
            m168_sb = csts_sb[:, STRIP:STRIP + 384]
            m24_sb = csts_sb[:, STRIP + 384:STRIP + 768]
            ident_sb = csts_sb[:, STRIP + 768:STRIP + 896]
            ones_sb = persist.tile([128, 1], bf16, tag="ones")
            nc.vector.memset(ones_sb, 1.0)
            eps_sb = persist.tile([128, 1], f32, tag="eps")
            nc.vector.memset(eps_sb, EPS)
            res16 = persist.tile([128, NBLK, D], f32, tag="res16")
            rsum16 = persist.tile([128, NBLK], f32, tag="rsum16")
            sqsum16 = persist.tile([128, NBLK], f32, tag="sqsum16")

            x_r = x_d[:].rearrange("(n p) d -> p n d", p=128)

            def xt_dma(q):
                for c in range(4):
                    nc.sync.dma_start(
                        out=xT_cq[c][q],
                        in_=x_d[:][q * 512:(q + 1) * 512, c * 128:(c + 1) * 128],
                        transpose=True,
                    )

            # critical-path order: wqk + xT(q0) gate the first projections,
            # wo gates xWo(0), masks/ident gate the first p1 blocks, x0 the
            # first residual.
            nc.sync.dma_start(
                out=wqk_sb, in_=wqk_d[:].rearrange("(c p) k -> p c k", p=128)
            )
            xt_dma(0)
            nc.sync.dma_start(
                out=wo_sb, in_=wo_d[:].rearrange("(c p) k -> p c k", p=128)
            )
            xt_dma(1)
            nc.sync.dma_start(out=csts_sb, in_=csts_d[:])
            nc.sync.dma_start(out=x_tiles[0], in_=x_r[:, 0:4, :])
            xt_dma(2)
            nc.sync.dma_start(out=x_tiles[1], in_=x_r[:, 4:8, :])
            xt_dma(3)
            nc.sync.dma_start(out=x_tiles[2], in_=x_r[:, 8:12, :])
            nc.sync.dma_start(out=x_tiles[3], in_=x_r[:, 12:16, :])
            if has_bq:
                bq_sb = persist.tile([128, 1], f32, tag="bq")
                nc.sync.dma_start(out=bq_sb, in_=bq_d[:])
            if has_bk:
                bk_sb = persist.tile([128, 1], f32, tag="bk")
                nc.sync.dma_start(out=bk_sb, in_=bk_d[:])
            if has_bo:
                ones1p_sb = persist.tile([1, 128], bf16, tag="ones1p")
                bo_sb = persist.tile([1, D], bf16, tag="bo")
                nc.sync.dma_start(out=ones1p_sb, in_=ones_d[:])
                nc.sync.dma_start(out=bo_sb, in_=bo_d[:])
            if has_gamma:
                gamma_sb = persist.tile([128, D], f32, tag="gamma")
                nc.sync.dma_start(out=gamma_sb, in_=gamma_d[:])
            if has_beta:
                beta_sb = persist.tile([128, D], f32, tag="beta")
                nc.sync.dma_start(out=beta_sb, in_=beta_d[:])

            with (
                tc.tile_pool(name="ps0", bufs=2, space="PSUM") as ps0,
                tc.tile_pool(name="s_ps", bufs=1, space="PSUM") as s_ps,
                tc.tile_pool(name="z_ps", bufs=1, space="PSUM") as z_ps,
                tc.tile_pool(name="ct_ps", bufs=1, space="PSUM") as ct_ps,
                tc.tile_pool(name="acc_ps", bufs=2, space="PSUM") as acc_ps,
                tc.tile_pool(name="work", bufs=2) as work,
                tc.tile_pool(name="small", bufs=3) as small,
            ):
                def p0_quarter(tq):
                    # qT / kT for this quarter
                    for idx, (w_sb, dst_q, bias_sb) in enumerate((
                        (wq_sb, qT_q, bq_sb if has_bq else None),
                        (wk_sb, kT_q, bk_sb if has_bk else None),
                    )):
                        pr = ps0.tile([128, 512], f32, tag="ps0", name="pr_ps")
                        for c in range(4):
                            nc.tensor.matmul(
                                out=pr,
                                lhsT=w_sb[:, c, :],
                                rhs=xT_cq[c][tq],
                                start=(c == 0),
                                stop=(c == 3),
                            )
                        if bias_sb is not None:
                            nc.scalar.activation(
                                out=dst_q[tq][:, :],
                                in_=pr,
                                func=AF.Identity,
                                bias=bias_sb[:, :],
                                scale=1.0,
                            )
                        else:
                            nc.scalar.activation(
                                out=dst_q[tq][:, :], in_=pr, func=AF.Copy
                            )

                def emit_xwo(ti):
                    # xWo for one row block (interleaved into pipeline steps
                    # so the PE has phase-0 filler during the p1 drain)
                    tq, tl = ti // 4, ti % 4
                    xw = ps0.tile([128, 512], f32, tag="ps0", name="xw_ps")
                    for c in range(4):
                        nc.tensor.matmul(
                            out=xw,
                            lhsT=xT_cq[c][tq][:, tl * 128:(tl + 1) * 128],
                            rhs=wo_sb[:, c, :],
                            start=(c == 0),
                            stop=(c == 3 and not has_bo),
                        )
                    if has_bo:
                        nc.tensor.matmul(
                            out=xw,
                            lhsT=ones1p_sb[:, :],
                            rhs=bo_sb[:, :],
                            start=False,
                            stop=True,
                        )
                    if ti % 4 == 0:
                        nc.vector.tensor_copy(out=xWo_q[tq][:, tl, :], in_=xw)
                    else:
                        nc.scalar.activation(
                            out=xWo_q[tq][:, tl, :], in_=xw, func=AF.Copy
                        )

                # per-block state for the software pipeline
                st = {}

                def geom(i):
                    jlo, jhi = max(0, i - HALO), min(NBLK - 1, i + HALO)
                    nb = jhi - jlo + 1
                    cb0 = jlo - i + HALO
                    mlo, mhi = max(0, i - 1), min(NBLK - 1, i + 1)
                    return jlo, jhi, nb, cb0, mlo, mhi

                def emit_sT_exp_masks(i):
                    jlo, jhi, nb, cb0, mlo, mhi = geom(i)
                    ncols = nb * 128
                    s_t = s_ps.tile([128, STRIP], f32, tag="s")
                    for p_ in range(nb):
                        j = jlo + p_
                        nc.tensor.matmul(
                            out=s_t[:, p_ * 128:(p_ + 1) * 128],
                            lhsT=kT_q[j // 4][:, (j % 4) * 128:(j % 4 + 1) * 128],
                            rhs=qT_q[i // 4][:, (i % 4) * 128:(i % 4 + 1) * 128],
                            start=True,
                            stop=True,
                        )
                    em = work.tile([128, STRIP], bf16, tag="em", bufs=6)
                    nc.scalar.activation(
                        out=em[:, :ncols], in_=s_t[:, :ncols], func=AF.Exp
                    )
                    # band mask: out-of-band parts of blocks |j-i| >= 2
                    lo_cols = (max(0, i - 1) - jlo) * 128
                    hi_cols = (min(NBLK - 1, i + 1) - jlo + 1) * 128
                    co = cb0 * 128
                    if lo_cols > 0:
                        nc.vector.tensor_mul(
                            out=em[:, :lo_cols],
                            in0=em[:, :lo_cols],
                            in1=band_sb[:, co:co + lo_cols],
                        )
                    if hi_cols < ncols:
                        nc.vector.tensor_mul(
                            out=em[:, hi_cols:ncols],
                            in0=em[:, hi_cols:ncols],
                            in1=band_sb[:, co + hi_cols:co + ncols],
                        )
                    # inner-window masked copies
                    ms = (mlo - jlo) * 128
                    mc = (mlo - i + 1) * 128
                    mcols = (mhi - mlo + 1) * 128
                    e168 = work.tile([128, 384], bf16, tag="e168", bufs=4)
                    e24 = work.tile([128, 384], bf16, tag="e24", bufs=4)
                    nc.gpsimd.tensor_mul(
                        out=e168[:, :mcols],
                        in0=em[:, ms:ms + mcols],
                        in1=m168_sb[:, mc:mc + mcols],
                    )
                    nc.gpsimd.tensor_mul(
                        out=e24[:, :mcols],
                        in0=em[:, ms:ms + mcols],
                        in1=m24_sb[:, mc:mc + mcols],
                    )
                    st[i] = dict(em=em, e168=e168, e24=e24)

                def emit_z(i):
                    jlo, jhi, nb, cb0, mlo, mhi = geom(i)
                    d = st[i]
                    z3 = z_ps.tile([128, 4], f32, tag="z3")
                    mcols = (mhi - mlo + 1) * 128
                    nm = mcols // 128
                    for p_ in range(nb):
                        nc.tensor.matmul(
                            out=z3[:, 0:1],
                            lhsT=d["em"][:, p_ * 128:(p_ + 1) * 128],
                            rhs=ones_sb,
                            start=(p_ == 0),
                            stop=(p_ == nb - 1),
                        )
                    for col, tname in ((1, "e168"), (2, "e24")):
                        for m_ in range(nm):
                            nc.tensor.matmul(
                                out=z3[:, col:col + 1],
                                lhsT=d[tname][:, m_ * 128:(m_ + 1) * 128],
                                rhs=ones_sb,
                                start=(m_ == 0),
                                stop=(m_ == nm - 1),
                            )
                    rcp = small.tile([128, 3], f32, tag="rcp", bufs=4)
                    nc.vector.reciprocal(out=rcp, in_=z3[:, 0:3])
                    z3s = small.tile([128, 1], f32, tag="z3s", bufs=2)
                    nc.vector.tensor_copy(out=z3s, in_=z3[:, 0:1])
                    cc = small.tile([128, 2], bf16, tag="cc", bufs=2)
                    nc.vector.tensor_scalar(
                        out=cc,
                        in0=rcp[:, 1:3],
                        scalar1=z3s[:, 0:1],
                        scalar2=None,
                        op0=OP.mult,
                    )
                    d["rcp"] = rcp
                    d["cc"] = cc

                def emit_ccchain(i):
                    # ccT transpose -> ccrow -> partition broadcast -> scale &
                    # merge inner windows into em (runs well before PV(i)).
                    jlo, jhi, nb, cb0, mlo, mhi = geom(i)
                    d = st[i]
                    # two single-row transposes: every consumer AP must
                    # start at partition 0 (BIR verifier rejects base=1)
                    ct = ct_ps.tile([1, 256], bf16, tag="ct", name="ct")
                    nc.tensor.matmul(
                        out=ct[:, 0:128],
                        lhsT=d["cc"][:, 0:1],
                        rhs=ident_sb,
                        is_transpose=True,
                        start=True,
                        stop=True,
                    )
                    nc.tensor.matmul(
                        out=ct[:, 128:256],
                        lhsT=d["cc"][:, 1:2],
                        rhs=ident_sb,
                        is_transpose=True,
                        start=True,
                        stop=True,
                    )
                    ccrow = small.tile([1, 256], bf16, tag="ccrow", bufs=2)
                    nc.vector.tensor_copy(out=ccrow, in_=ct)
                    ccb = small.tile([128, 256], bf16, tag="ccb", bufs=2)
                    nc.gpsimd.partition_broadcast(ccb[:, 0:128], ccrow[:, 0:128])
                    nc.gpsimd.partition_broadcast(ccb[:, 128:256], ccrow[:, 128:256])
                    ms = (mlo - jlo) * 128
                    mcols = (mhi - mlo + 1) * 128
                    nm = mcols // 128
                    em, e168, e24 = d["em"], d["e168"], d["e24"]
                    for tname, coff in (("e168", 0), ("e24", 128)):
                        tl = d[tname]
                        t3 = tl[:, :mcols].rearrange("p (m t) -> p m t", m=nm)
                        cb = ccb[:, coff:coff + 128]
                        cb3 = bass.AP(
                            tensor=cb.tensor,
                            offset=cb.offset,
                            ap=[cb.ap[0], [0, nm], [1, 128]],
                        )
                        nc.vector.tensor_mul(out=t3, in0=t3, in1=cb3)
                    nc.vector.tensor_add(
                        out=em[:, ms:ms + mcols],
                        in0=em[:, ms:ms + mcols],
                        in1=e168[:, :mcols],
                    )
                    nc.vector.tensor_add(
                        out=em[:, ms:ms + mcols],
                        in0=em[:, ms:ms + mcols],
                        in1=e24[:, :mcols],
                    )

                def emit_pv(i, part):
                    jlo, jhi, nb, cb0, mlo, mhi = geom(i)
                    d = st[i]
                    if part == 0:
                        d["acc"] = acc_ps.tile([128, 512], f32, tag="acc", name="acc_ps")
                        rng = range(0, min(6, nb))
                    else:
                        rng = range(min(6, nb), nb)
                    for p_ in rng:
                        j = jlo + p_
                        nc.tensor.matmul(
                            out=d["acc"],
                            lhsT=d["em"][:, p_ * 128:(p_ + 1) * 128],
                            rhs=xWo_q[j // 4][:, j % 4, :],
                            start=(p_ == 0),
                            stop=(p_ == nb - 1),
                        )

                def emit_res(i):
                    d = st[i]
                    # res = acc/z720 + x ; row-sum for LN mean (on gpsimd)
                    nc.vector.scalar_tensor_tensor(
                        out=res16[:, i, :],
                        in0=d["acc"],
                        scalar=d["rcp"][:, 0:1],
                        in1=x_tiles[i // 4][:, i % 4, :],
                        op0=OP.mult,
                        op1=OP.add,
                        accum_out=rsum16[:, i:i + 1],
                    )
                    sqscr = work.tile([128, D], f32, tag="sqscr", bufs=2)
                    nc.scalar.activation(
                        out=sqscr,
                        in_=res16[:, i, :],
                        func=AF.Square,
                        accum_out=sqsum16[:, i:i + 1],
                    )
                    del st[i]["acc"]

                def ln_quarter(q, h0=None, hn=4):
                    if h0 is None:
                        h0 = 4 * q
                    hsl = slice(h0, h0 + hn)
                    mu = small.tile([128, hn], f32, tag="mu", bufs=3)
                    nc.vector.tensor_scalar_mul(
                        out=mu, in0=rsum16[:, hsl], scalar1=1.0 / D
                    )
                    musq = small.tile([128, hn], f32, tag="musq", bufs=3)
                    nc.vector.tensor_mul(out=musq, in0=mu, in1=mu)
                    var = small.tile([128, hn], f32, tag="var", bufs=3)
                    nc.vector.tensor_scalar(
                        out=var,
                        in0=sqsum16[:, hsl],
                        scalar1=1.0 / D,
                        scalar2=EPS,
                        op0=OP.mult,
                        op1=OP.add,
                    )
                    nc.vector.tensor_sub(out=var, in0=var, in1=musq)
                    # rstd = 1/sqrt(var) via DVE-only Newton iteration (keeps
                    # the ACT table on exp for the whole kernel): seed from the
                    # hardware reciprocal, y0 = (1+r)/2 ~ sqrt(r), then two
                    # steps of y <- y*(1.5 - 0.5*var*y^2).
                    rv = small.tile([128, hn], f32, tag="rv", bufs=3)
                    nc.vector.reciprocal(out=rv, in_=var)
                    rstd = small.tile([128, hn], f32, tag="rstd", bufs=3)
                    nc.vector.tensor_scalar(
                        out=rstd,
                        in0=rv,
                        scalar1=0.5,
                        scalar2=0.5,
                        op0=OP.mult,
                        op1=OP.add,
                    )
                    u = small.tile([128, hn], f32, tag="u", bufs=3)
                    for _ in range(2):
                        nc.vector.tensor_mul(out=u, in0=rstd, in1=rstd)
                        nc.vector.tensor_mul(out=u, in0=u, in1=var)
                        nc.vector.tensor_scalar(
                            out=u,
                            in0=u,
                            scalar1=-0.5,
                            scalar2=1.5,
                            op0=OP.mult,
                            op1=OP.add,
                        )
                        nc.vector.tensor_mul(out=rstd, in0=rstd, in1=u)
                    nmb = small.tile([128, hn], f32, tag="nmb", bufs=3)
                    # nmb = -mu * rstd (elementwise across the 4 block columns)
                    nc.vector.scalar_tensor_tensor(
                        out=nmb,
                        in0=mu,
                        scalar=-1.0,
                        in1=rstd,
                        op0=OP.mult,
                        op1=OP.mult,
                    )
                    outq = work.tile([128, hn, D], bf16, tag="outq", bufs=3)
                    for k in range(hn):
                        ib = h0 + k
                        if k == 1:
                            nc.scalar.activation(
                                out=outq[:, k, :],
                                in_=res16[:, ib, :],
                                func=AF.Identity,
                                bias=nmb[:, k:k + 1],
                                scale=rstd[:, k:k + 1],
                            )
                        elif k == 3:
                            nc.scalar.activation(
                                out=outq[:, k, :],
                                in_=res16[:, ib, :],
                                func=AF.Identity,
                                bias=nmb[:, k:k + 1],
                                scale=rstd[:, k:k + 1],
                            )
                        else:
                            nc.vector.tensor_scalar(
                                out=outq[:, k, :],
                                in0=res16[:, ib, :],
                                scalar1=rstd[:, k:k + 1],
                                scalar2=nmb[:, k:k + 1],
                                op0=OP.mult,
                                op1=OP.add,
                            )
                        if has_gamma:
                            nc.gpsimd.tensor_mul(
                                out=outq[:, k, :], in0=outq[:, k, :], in1=gamma_sb
                            )
                        if has_beta:
                            nc.gpsimd.tensor_add(
                                out=outq[:, k, :], in0=outq[:, k, :], in1=beta_sb
                            )
                    out_r = out_d[:].rearrange("(n p) d -> p n d", p=128)
                    nc.sync.dma_start(
                        out=out_r[:, h0:h0 + hn, :], in_=outq
                    )

                LAG = 3

                def pipeline_step(i):
                    """Emit work for pipeline step i (i in 0..NBLK+LAG-1)."""
                    if i + HALO < NBLK:
                        emit_xwo(i + HALO)
                    if i < NBLK:
                        emit_sT_exp_masks(i)
                    if i - LAG >= 0:
                        emit_pv(i - LAG, 0)
                    if 0 <= i - 1 < NBLK:
                        emit_z(i - 1)
                    if i - LAG >= 0:
                        emit_pv(i - LAG, 1)
                        emit_res(i - LAG)
                    if 0 <= i - 1 < NBLK:
                        emit_ccchain(i - 1)
                    if i - LAG >= 0 and (i - LAG) % 2 == 1:
                        ln_quarter(None, h0=i - LAG - 1, hn=2)

                done = 0
                for tq in range(4):
                    p0_quarter(tq)
                    if tq == 0:
                        for ti in range(HALO):
                            emit_xwo(ti)
                    while done < NBLK and (min(done + HALO, NBLK - 1)) // 4 <= tq:
                        pipeline_step(done)
                        done += 1
                while done < NBLK + LAG:
                    pipeline_step(done)
                    done += 1

    nc.compile()
    return nc


def _get_built(flags):
    if flags not in _CACHE:
        _CACHE[flags] = _build_nc(*flags)
    return _CACHE[flags]


def _make_in_maps(x, Wq, bq, Wk, bk, Wo, bo, gamma, beta, flags):
    import ml_dtypes

    bf = ml_dtypes.bfloat16
    has_bq, has_bk, has_bo, has_gamma, has_beta = flags
    band01T, m168T, m24T, ident = _host_consts()
    scale = 1.0 / math.sqrt(DK)
    wqk = np.concatenate([(Wq * scale).astype(bf), Wk.astype(bf)], axis=1)
    csts = np.concatenate([band01T, m168T, m24T, ident], axis=1)
    base = {
        "Wqk_s": np.ascontiguousarray(wqk),
        "Wo": np.ascontiguousarray((Wo / 3.0).astype(bf)),
        "csts": np.ascontiguousarray(csts),
    }
    if has_bq:
        base["bq_s"] = np.ascontiguousarray(bq * scale, dtype=np.float32).reshape(DK, 1)
    if has_bk:
        base["bk_c"] = np.ascontiguousarray(bk, dtype=np.float32).reshape(DK, 1)
    if has_bo:
        base["ones_row"] = np.ones((1, 128), dtype=np.float32).astype(bf)
        base["bo_row"] = (bo / 3.0).astype(bf).reshape(1, D)
    if has_gamma:
        base["gamma_bc"] = np.broadcast_to(
            np.asarray(gamma, dtype=np.float32), (128, D)
        ).copy()
    if has_beta:
        base["beta_bc"] = np.broadcast_to(
            np.asarray(beta, dtype=np.float32), (128, D)
        ).copy()
    xb = np.ascontiguousarray(x).astype(bf)
    return [{**base, "x": xb[core]} for core in range(B)]


def kernel(x, Wq, bq, Wk, bk, Wo, bo, gamma, beta):
    from concourse.bass_utils import run_bass_kernel_spmd

    x = np.asarray(x, dtype=np.float32)
    Wq = np.asarray(Wq, dtype=np.float32)
    bq = np.asarray(bq, dtype=np.float32)
    Wk = np.asarray(Wk, dtype=np.float32)
    bk = np.asarray(bk, dtype=np.float32)
    Wo = np.asarray(Wo, dtype=np.float32)
    bo = np.asarray(bo, dtype=np.float32)
    gamma = np.asarray(gamma, dtype=np.float32)
    beta = np.asarray(beta, dtype=np.float32)

    flags = (
        bool(np.any(bq != 0.0)),
        bool(np.any(bk != 0.0)),
        bool(np.any(bo != 0.0)),
        bool(np.any(gamma != 1.0)),
        bool(np.any(beta != 0.0)),
    )
    nc = _get_built(flags)
    in_maps = _make_in_maps(x, Wq, bq, Wk, bk, Wo, bo, gamma, beta, flags)
    res = run_bass_kernel_spmd(nc, in_maps, list(range(B)))
    return np.stack(
        [np.asarray(res.results[c]["out"], dtype=np.float32) for c in range(B)], axis=0
    )


# revision 5
# speedup vs baseline: 1.0064x; 1.0055x over previous
"""Trainium2 Bass kernel for nn_AttentionTemporelle (3-window banded attention).

v2: transposed-scores design, bf16 datapath, DMA-XBAR transposes.

Per batch element (data-parallel over B=8, one per core):
    q = x @ Wq ; k = x @ Wk                     [T, DK]
    s = q k^T / sqrt(DK); 3 banded softmaxes averaged; @x; @Wo; +x; LayerNorm

Key structure (vs v1):
  * Scores computed TRANSPOSED per 128-row block i: for each j-block in the
    +-3-block strip, sT[j,t] = matmul(lhsT=kT_j, rhs=qT_i) -> psum [128, nb*128].
    exp() writes em[j,t] straight to SBUF bf16 -- the PV matmul consumes em as
    lhsT directly, so v1's 13 PE transposes + psum->sbuf copy per block vanish.
  * Window merge: em_mid += e168*c168(t) + e24*c24(t), where the per-t scales
    c168 = Z720/Z168 etc. Z's come from N=1 matmuls (lhsT=em tiles, rhs=ones),
    the scale row-vector from one tiny PE transpose + gpsimd partition
    broadcasts. One PV (7 matmuls vs x@Wo/3) then computes all three windows.
  * rstd = 1/sqrt(var+eps) via DVE reciprocal + two Newton steps: no Sqrt on
    the ACT engine, so one act-table load (exp set) serves the whole kernel.
  * bf16 everywhere on the PE/DVE path (fp32 psum accumulation), bf16 HBM I/O,
    xT via DMA XBAR transpose (16 chunk transposes, no PE work).
  * LayerNorm folded per quarter; output DMA'd per quarter (bf16).
  * PE emission is software-pipelined with a 3-step PV lag: step i emits
    xWo(i+3), sT(i), PV(i-3) part A, Z(i-1), PV(i-3) part B, ccT(i-1) so
    cross-engine chains get ~3 block periods of slack before PV consumes
    their results and the PE queue rarely drains.
"""

import math

import numpy as np

B, T, D, DK = 8, 2048, 512, 128
NBLK = T // 128                 # 16 row blocks
HALO = 3                        # 360 // 128 + 1 neighbor blocks each side
STRIP = (2 * HALO + 1) * 128    # 896
EPS = 1e-5
H720, H168, H24 = 360, 84, 12

_CACHE = {}


def _host_consts():
    import ml_dtypes

    bf = ml_dtypes.bfloat16
    p = np.arange(128)[:, None, None]
    jb7 = np.arange(7)[None, :, None]
    tt = np.arange(128)[None, None, :]
    d7 = (jb7 - HALO) * 128 + p - tt          # j_global - t, canonical strip
    band01T = (np.abs(d7) <= H720).astype(bf).reshape(128, STRIP)
    jb3 = np.arange(3)[None, :, None]
    d3 = (jb3 - 1) * 128 + p - tt
    m168T = (np.abs(d3) <= H168).astype(bf).reshape(128, 384)
    m24T = (np.abs(d3) <= H24).astype(bf).reshape(128, 384)
    ident = np.eye(128, dtype=np.float32).astype(bf)
    return band01T, m168T, m24T, ident


def _build_nc(has_bq, has_bk, has_bo, has_gamma, has_beta):
    import concourse.bass as bass
    import concourse.tile as tile
    from concourse import bacc, mybir

    f32 = mybir.dt.float32
    bf16 = mybir.dt.bfloat16
    AF = mybir.ActivationFunctionType
    OP = mybir.AluOpType

    nc = bacc.Bacc()

    x_d = nc.declare_dram_parameter("x", [T, D], bf16, isOutput=False)
    wqk_d = nc.declare_dram_parameter("Wqk_s", [D, 2 * DK], bf16, isOutput=False)
    wo_d = nc.declare_dram_parameter("Wo", [D, D], bf16, isOutput=False)
    # band01T | m168T | m24T | ident packed along the free dim (one DMA)
    csts_d = nc.declare_dram_parameter(
        "csts", [128, STRIP + 384 + 384 + 128], bf16, isOutput=False
    )
    if has_bq:
        bq_d = nc.declare_dram_parameter("bq_s", [DK, 1], f32, isOutput=False)
    if has_bk:
        bk_d = nc.declare_dram_parameter("bk_c", [DK, 1], f32, isOutput=False)
    if has_bo:
        ones_d = nc.declare_dram_parameter("ones_row", [1, 128], bf16, isOutput=False)
        bo_d = nc.declare_dram_parameter("bo_row", [1, D], bf16, isOutput=False)
    if has_gamma:
        gamma_d = nc.declare_dram_parameter("gamma_bc", [128, D], f32, isOutput=False)
    if has_beta:
        beta_d = nc.declare_dram_parameter("beta_bc", [128, D], f32, isOutput=False)
    out_d = nc.declare_dram_parameter("out", [T, D], bf16, isOutput=True)

    with tile.TileContext(nc) as tc:
        with tc.tile_pool(name="persist", bufs=1) as persist:
            x_tiles = [
                persist.tile([128, 4, D], bf16, tag=f"x{g}", name=f"x_sb{g}")
                for g in range(4)
            ]
            # per-(chunk, quarter) tiles: a single [128, T] tile per chunk
            # false-shares across quarters (q1's transpose DMA would wait on
            # every q0 reader), so keep them separate.
            xT_cq = [
                [
                    persist.tile(
                        [128, 512], bf16, tag=f"xT{c}_{q}", name=f"xT_sb{c}_{q}"
                    )
                    for q in range(4)
                ]
                for c in range(4)
            ]
            qT_q = [
                persist.tile([128, 512], bf16, tag=f"qT{g}", name=f"qT_sb{g}")
                for g in range(4)
            ]
            kT_q = [
                persist.tile([128, 512], bf16, tag=f"kT{g}", name=f"kT_sb{g}")
                for g in range(4)
            ]
            xWo_q = [
                persist.tile([128, 4, D], bf16, tag=f"xWo{g}", name=f"xWo_sb{g}")
                for g in range(4)
            ]
            wqk_sb = persist.tile([128, 4, 2 * DK], bf16, tag="wqk")
            wq_sb = wqk_sb[:, :, 0:DK]
            wk_sb = wqk_sb[:, :, DK:2 * DK]
            wo_sb = persist.tile([128, 4, D], bf16, tag="wo")
            csts_sb = persist.tile(
                [128, STRIP + 384 + 384 + 128], bf16, tag="csts"
            )
            band_sb = csts_sb[:, 0:STRIP]
            m168_sb = csts_sb[:, STRIP:STRIP + 384]
            m24_sb = csts_sb[:, STRIP + 384:STRIP + 768]
            ident_sb = csts_sb[:, STRIP + 768:STRIP + 896]
            ones_sb = persist.tile([128, 1], bf16, tag="ones")
            nc.vector.memset(ones_sb, 1.0)
            eps_sb = persist.tile([128, 1], f32, tag="eps")
            nc.vector.memset(eps_sb, EPS)
            res16 = persist.tile([128, NBLK, D], f32, tag="res16")
            rsum16 = persist.tile([128, NBLK], f32, tag="rsum16")
            sqsum16 = persist.tile([128, NBLK], f32, tag="sqsum16")

            x_r = x_d[:].rearrange("(n p) d -> p n d", p=128)

            def xt_dma(q):
                for c in range(4):
                    nc.sync.dma_start(
                        out=xT_cq[c][q],
                        in_=x_d[:][q * 512:(q + 1) * 512, c * 128:(c + 1) * 128],
                        transpose=True,
                    )

            # critical-path order: wqk + xT(q0) gate the first projections,
            # wo gates xWo(0), masks/ident gate the first p1 blocks, x0 the
            # first residual.
            nc.sync.dma_start(
                out=wqk_sb, in_=wqk_d[:].rearrange("(c p) k -> p c k", p=128)
            )
            xt_dma(0)
            nc.sync.dma_start(
                out=wo_sb, in_=wo_d[:].rearrange("(c p) k -> p c k", p=128)
            )
            xt_dma(1)
            nc.sync.dma_start(out=csts_sb, in_=csts_d[:])
            nc.sync.dma_start(out=x_tiles[0], in_=x_r[:, 0:4, :])
            xt_dma(2)
            nc.sync.dma_start(out=x_tiles[1], in_=x_r[:, 4:8, :])
            xt_dma(3)
            nc.sync.dma_start(out=x_tiles[2], in_=x_r[:, 8:12, :])
            nc.sync.dma_start(out=x_tiles[3], in_=x_r[:, 12:16, :])
            if has_bq:
                bq_sb = persist.tile([128, 1], f32, tag="bq")
                nc.sync.dma_start(out=bq_sb, in_=bq_d[:])
            if has_bk:
                bk_sb = persist.tile([128, 1], f32, tag="bk")
                nc.sync.dma_start(out=bk_sb, in_=bk_d[:])
            if has_bo:
                ones1p_sb = persist.tile([1, 128], bf16, tag="ones1p")
                bo_sb = persist.tile([1, D], bf16, tag="bo")
                nc.sync.dma_start(out=ones1p_sb, in_=ones_d[:])
                nc.sync.dma_start(out=bo_sb, in_=bo_d[:])
            if has_gamma:
                gamma_sb = persist.tile([128, D], f32, tag="gamma")
                nc.sync.dma_start(out=gamma_sb, in_=gamma_d[:])
            if has_beta:
                beta_sb = persist.tile([128, D], f32, tag="beta")
                nc.sync.dma_start(out=beta_sb, in_=beta_d[:])

            with (
                tc.tile_pool(name="ps0", bufs=2, space="PSUM") as ps0,
                tc.tile_pool(name="s_ps", bufs=1, space="PSUM") as s_ps,
                tc.tile_pool(name="z_ps", bufs=1, space="PSUM") as z_ps,
                tc.tile_pool(name="ct_ps", bufs=1, space="PSUM") as ct_ps,
                tc.tile_pool(name="acc_ps", bufs=2, space="PSUM") as acc_ps,
                tc.tile_pool(name="work", bufs=2) as work,
                tc.tile_pool(name="small", bufs=3) as small,
            ):
                def p0_quarter(tq):
                    # qT / kT for this quarter
                    for idx, (w_sb, dst_q, bias_sb) in enumerate((
                        (wq_sb, qT_q, bq_sb if has_bq else None),
                        (wk_sb, kT_q, bk_sb if has_bk else None),
                    )):
                        pr = ps0.tile([128, 512], f32, tag="ps0", name="pr_ps")
                        for c in range(4):
                            nc.tensor.matmul(
                                out=pr,
                                lhsT=w_sb[:, c, :],
                                rhs=xT_cq[c][tq],
                                start=(c == 0),
                                stop=(c == 3),
                            )
                        if bias_sb is not None:
                            nc.scalar.activation(
                                out=dst_q[tq][:, :],
                                in_=pr,
                                func=AF.Identity,
                                bias=bias_sb[:, :],
                                scale=1.0,
                            )
                        else:
                            nc.scalar.activation(
                                out=dst_q[tq][:, :], in_=pr, func=AF.Copy
                            )

                def emit_xwo(ti):
                    # xWo for one row block (interleaved into pipeline steps
                    # so the PE has phase-0 filler during the p1 drain)
                    tq, tl = ti // 4, ti % 4
                    xw = ps0.tile([128, 512], f32, tag="ps0", name="xw_ps")
                    for c in range(4):
                        nc.tensor.matmul(
                            out=xw,
                            lhsT=xT_cq[c][tq][:, tl * 128:(tl + 1) * 128],
                            rhs=wo_sb[:, c, :],
                            start=(c == 0),
                            stop=(c == 3 and not has_bo),
                        )
                    if has_bo:
                        nc.tensor.matmul(
                            out=xw,
                            lhsT=ones1p_sb[:, :],
                            rhs=bo_sb[:, :],
                            start=False,
                            stop=True,
                        )
                    if ti % 4 == 0:
                        nc.vector.tensor_copy(out=xWo_q[tq][:, tl, :], in_=xw)
                    else:
                        nc.scalar.activation(
                            out=xWo_q[tq][:, tl, :], in_=xw, func=AF.Copy
                        )

                # per-block state for the software pipeline
                st = {}

                def geom(i):
                    jlo, jhi = max(0, i - HALO), min(NBLK - 1, i + HALO)
                    nb = jhi - jlo + 1
                    cb0 = jlo - i + HALO
                    mlo, mhi = max(0, i - 1), min(NBLK - 1, i + 1)
                    return jlo, jhi, nb, cb0, mlo, mhi

                def emit_sT_exp_masks(i):
                    jlo, jhi, nb, cb0, mlo, mhi = geom(i)
                    ncols = nb * 128
                    s_t = s_ps.tile([128, STRIP], f32, tag="s")
                    for p_ in range(nb):
                        j = jlo + p_
                        nc.tensor.matmul(
                            out=s_t[:, p_ * 128:(p_ + 1) * 128],
                            lhsT=kT_q[j // 4][:, (j % 4) * 128:(j % 4 + 1) * 128],
                            rhs=qT_q[i // 4][:, (i % 4) * 128:(i % 4 + 1) * 128],
                            start=True,
                            stop=True,
                        )
                    em = work.tile([128, STRIP], bf16, tag="em", bufs=6)
                    nc.scalar.activation(
                        out=em[:, :ncols], in_=s_t[:, :ncols], func=AF.Exp
                    )
                    # band mask: out-of-band parts of blocks |j-i| >= 2
                    lo_cols = (max(0, i - 1) - jlo) * 128
                    hi_cols = (min(NBLK - 1, i + 1) - jlo + 1) * 128
                    co = cb0 * 128
                    if lo_cols > 0:
                        nc.vector.tensor_mul(
                            out=em[:, :lo_cols],
                            in0=em[:, :lo_cols],
                            in1=band_sb[:, co:co + lo_cols],
                        )
                    if hi_cols < ncols:
                        nc.vector.tensor_mul(
                            out=em[:, hi_cols:ncols],
                            in0=em[:, hi_cols:ncols],
                            in1=band_sb[:, co + hi_cols:co + ncols],
                        )
                    # inner-window masked copies
                    ms = (mlo - jlo) * 128
                    mc = (mlo - i + 1) * 128
                    mcols = (mhi - mlo + 1) * 128
                    e168 = work.tile([128, 384], bf16, tag="e168", bufs=4)
                    e24 = work.tile([128, 384], bf16, tag="e24", bufs=4)
                    nc.gpsimd.tensor_mul(
                        out=e168[:, :mcols],
                        in0=em[:, ms:ms + mcols],
                        in1=m168_sb[:, mc:mc + mcols],
                    )
                    nc.gpsimd.tensor_mul(
                        out=e24[:, :mcols],
                        in0=em[:, ms:ms + mcols],
                        in1=m24_sb[:, mc:mc + mcols],
                    )
                    st[i] = dict(em=em, e168=e168, e24=e24)

                def emit_z(i):
                    jlo, jhi, nb, cb0, mlo, mhi = geom(i)
                    d = st[i]
                    z3 = z_ps.tile([128, 4], f32, tag="z3")
                    mcols = (mhi - mlo + 1) * 128
                    nm = mcols // 128
                    for p_ in range(nb):
                        nc.tensor.matmul(
                            out=z3[:, 0:1],
                            lhsT=d["em"][:, p_ * 128:(p_ + 1) * 128],
                            rhs=ones_sb,
                            start=(p_ == 0),
                            stop=(p_ == nb - 1),
                        )
                    for col, tname in ((1, "e168"), (2, "e24")):
                        for m_ in range(nm):
                            nc.tensor.matmul(
                                out=z3[:, col:col + 1],
                                lhsT=d[tname][:, m_ * 128:(m_ + 1) * 128],
                                rhs=ones_sb,
                                start=(m_ == 0),
                                stop=(m_ == nm - 1),
                            )
                    rcp = small.tile([128, 3], f32, tag="rcp", bufs=4)
                    nc.vector.reciprocal(out=rcp, in_=z3[:, 0:3])
                    z3s = small.tile([128, 1], f32, tag="z3s", bufs=2)
                    nc.vector.tensor_copy(out=z3s, in_=z3[:, 0:1])
                    cc = small.tile([128, 2], bf16, tag="cc", bufs=2)
                    nc.vector.tensor_scalar(
                        out=cc,
                        in0=rcp[:, 1:3],
                        scalar1=z3s[:, 0:1],
                        scalar2=None,
                        op0=OP.mult,
                    )
                    d["rcp"] = rcp
                    d["cc"] = cc

                def emit_ccchain(i):
                    # ccT transpose -> ccrow -> partition broadcast -> scale &
                    # merge inner windows into em (runs well before PV(i)).
                    # high_priority hoists these latency-critical ops ahead of
                    # bulk res/LN work in the in-order DVE queue.
                    jlo, jhi, nb, cb0, mlo, mhi = geom(i)
                    d = st[i]
                    # two single-row transposes: every consumer AP must
                    # start at partition 0 (BIR verifier rejects base=1)
                    ct = ct_ps.tile([1, 256], bf16, tag="ct", name="ct")
                    nc.tensor.matmul(
                        out=ct[:, 0:128],
                        lhsT=d["cc"][:, 0:1],
                        rhs=ident_sb,
                        is_transpose=True,
                        start=True,
                        stop=True,
                    )
                    nc.tensor.matmul(
                        out=ct[:, 128:256],
                        lhsT=d["cc"][:, 1:2],
                        rhs=ident_sb,
                        is_transpose=True,
                        start=True,
                        stop=True,
                    )
                    ccrow = small.tile([1, 256], bf16, tag="ccrow", bufs=2)
                    nc.vector.tensor_copy(out=ccrow, in_=ct)
                    ccb = small.tile([128, 256], bf16, tag="ccb", bufs=2)
                    nc.gpsimd.partition_broadcast(ccb[:, 0:128], ccrow[:, 0:128])
                    nc.gpsimd.partition_broadcast(ccb[:, 128:256], ccrow[:, 128:256])
                    ms = (mlo - jlo) * 128
                    mcols = (mhi - mlo + 1) * 128
                    nm = mcols // 128
                    em, e168, e24 = d["em"], d["e168"], d["e24"]
                    for tname, coff in (("e168", 0), ("e24", 128)):
                        tl = d[tname]
                        t3 = tl[:, :mcols].rearrange("p (m t) -> p m t", m=nm)
                        cb = ccb[:, coff:coff + 128]
                        cb3 = bass.AP(
                            tensor=cb.tensor,
                            offset=cb.offset,
                            ap=[cb.ap[0], [0, nm], [1, 128]],
                        )
                        nc.vector.tensor_mul(out=t3, in0=t3, in1=cb3)
                    nc.vector.tensor_add(
                        out=em[:, ms:ms + mcols],
                        in0=em[:, ms:ms + mcols],
                        in1=e168[:, :mcols],
                    )
                    nc.vector.tensor_add(
                        out=em[:, ms:ms + mcols],
                        in0=em[:, ms:ms + mcols],
                        in1=e24[:, :mcols],
                    )

                def emit_pv(i, part):
                    jlo, jhi, nb, cb0, mlo, mhi = geom(i)
                    d = st[i]
                    if part == 0:
                        d["acc"] = acc_ps.tile([128, 512], f32, tag="acc", name="acc_ps")
                        rng = range(0, min(6, nb))
                    else:
                        rng = range(min(6, nb), nb)
                    for p_ in rng:
                        j = jlo + p_
                        nc.tensor.matmul(
                            out=d["acc"],
                            lhsT=d["em"][:, p_ * 128:(p_ + 1) * 128],
                            rhs=xWo_q[j // 4][:, j % 4, :],
                            start=(p_ == 0),
                            stop=(p_ == nb - 1),
                        )

                def emit_res(i):
                    d = st[i]
                    # res = acc/z720 + x ; row-sum for LN mean (on gpsimd)
                    nc.vector.scalar_tensor_tensor(
                        out=res16[:, i, :],
                        in0=d["acc"],
                        scalar=d["rcp"][:, 0:1],
                        in1=x_tiles[i // 4][:, i % 4, :],
                        op0=OP.mult,
                        op1=OP.add,
                        accum_out=rsum16[:, i:i + 1],
                    )
                    sqscr = work.tile([128, D], f32, tag="sqscr", bufs=2)
                    nc.scalar.activation(
                        out=sqscr,
                        in_=res16[:, i, :],
                        func=AF.Square,
                        accum_out=sqsum16[:, i:i + 1],
                    )
                    del st[i]["acc"]

                def ln_quarter(q, h0=None, hn=4):
                    if h0 is None:
                        h0 = 4 * q
                    hsl = slice(h0, h0 + hn)
                    mu = small.tile([128, hn], f32, tag="mu", bufs=3)
                    nc.vector.tensor_scalar_mul(
                        out=mu, in0=rsum16[:, hsl], scalar1=1.0 / D
                    )
                    musq = small.tile([128, hn], f32, tag="musq", bufs=3)
                    nc.vector.tensor_mul(out=musq, in0=mu, in1=mu)
                    var = small.tile([128, hn], f32, tag="var", bufs=3)
                    nc.vector.tensor_scalar(
                        out=var,
                        in0=sqsum16[:, hsl],
                        scalar1=1.0 / D,
                        scalar2=EPS,
                        op0=OP.mult,
                        op1=OP.add,
                    )
                    nc.vector.tensor_sub(out=var, in0=var, in1=musq)
                    # rstd = 1/sqrt(var) via DVE-only Newton iteration (keeps
                    # the ACT table on exp for the whole kernel): seed from the
                    # hardware reciprocal, y0 = (1+r)/2 ~ sqrt(r), then two
                    # steps of y <- y*(1.5 - 0.5*var*y^2).
                    rv = small.tile([128, hn], f32, tag="rv", bufs=3)
                    nc.vector.reciprocal(out=rv, in_=var)
                    rstd = small.tile([128, hn], f32, tag="rstd", bufs=3)
                    nc.vector.tensor_scalar(
                        out=rstd,
                        in0=rv,
                        scalar1=0.5,
                        scalar2=0.5,
                        op0=OP.mult,
                        op1=OP.add,
                    )
                    u = small.tile([128, hn], f32, tag="u", bufs=3)
                    for _ in range(2):
                        nc.vector.tensor_mul(out=u, in0=rstd, in1=rstd)
                        nc.vector.tensor_mul(out=u, in0=u, in1=var)
                        nc.vector.tensor_scalar(
                            out=u,
                            in0=u,
                            scalar1=-0.5,
                            scalar2=1.5,
                            op0=OP.mult,
                            op1=OP.add,
                        )
                        nc.vector.tensor_mul(out=rstd, in0=rstd, in1=u)
                    nmb = small.tile([128, hn], f32, tag="nmb", bufs=3)
                    # nmb = -mu * rstd (elementwise across the 4 block columns)
                    nc.vector.scalar_tensor_tensor(
                        out=nmb,
                        in0=mu,
                        scalar=-1.0,
                        in1=rstd,
                        op0=OP.mult,
                        op1=OP.mult,
                    )
                    outq = work.tile([128, hn, D], bf16, tag="outq", bufs=3)
                    for k in range(hn):
                        ib = h0 + k
                        if k == 1:
                            nc.scalar.activation(
                                out=outq[:, k, :],
                                in_=res16[:, ib, :],
                                func=AF.Identity,
                                bias=nmb[:, k:k + 1],
                                scale=rstd[:, k:k + 1],
                            )
                        elif k == 3:
                            nc.scalar.activation(
                                out=outq[:, k, :],
                                in_=res16[:, ib, :],
                                func=AF.Identity,
                                bias=nmb[:, k:k + 1],
                                scale=rstd[:, k:k + 1],
                            )
                        else:
                            nc.vector.tensor_scalar(
                                out=outq[:, k, :],
                                in0=res16[:, ib, :],
                                scalar1=rstd[:, k:k + 1],
                                scalar2=nmb[:, k:k + 1],
                                op0=OP.mult,
                                op1=OP.add,
                            )
                        if has_gamma:
                            nc.gpsimd.tensor_mul(
                                out=outq[:, k, :], in0=outq[:, k, :], in1=gamma_sb
                            )
                        if has_beta:
                            nc.gpsimd.tensor_add(
                                out=outq[:, k, :], in0=outq[:, k, :], in1=beta_sb
                            )
                    out_r = out_d[:].rearrange("(n p) d -> p n d", p=128)
                    nc.sync.dma_start(
                        out=out_r[:, h0:h0 + hn, :], in_=outq
                    )

                LAG = 3

                def pipeline_step(i):
                    """Emit work for pipeline step i (i in 0..NBLK+LAG-1)."""
                    if i + HALO < NBLK:
                        emit_xwo(i + HALO)
                    if i < NBLK:
                        emit_sT_exp_masks(i)
                    if i - LAG >= 0:
                        emit_pv(i - LAG, 0)
                    if 0 <= i - 1 < NBLK:
                        emit_z(i - 1)
                    if i - LAG >= 0:
                        emit_pv(i - LAG, 1)
                        emit_res(i - LAG)
                    if 0 <= i - 1 < NBLK:
                        with tc.high_priority(offset=60):
                            emit_ccchain(i - 1)
                    if i - LAG >= 0 and (i - LAG) % 2 == 1:
                        ln_quarter(None, h0=i - LAG - 1, hn=2)

                done = 0
                for tq in range(4):
                    p0_quarter(tq)
                    if tq == 0:
                        for ti in range(HALO):
                            emit_xwo(ti)
                    while done < NBLK and (min(done + HALO, NBLK - 1)) // 4 <= tq:
                        pipeline_step(done)
                        done += 1
                while done < NBLK + LAG:
                    pipeline_step(done)
                    done += 1

    nc.compile()
    return nc


def _get_built(flags):
    if flags not in _CACHE:
        _CACHE[flags] = _build_nc(*flags)
    return _CACHE[flags]


def _make_in_maps(x, Wq, bq, Wk, bk, Wo, bo, gamma, beta, flags):
    import ml_dtypes

    bf = ml_dtypes.bfloat16
    has_bq, has_bk, has_bo, has_gamma, has_beta = flags
    band01T, m168T, m24T, ident = _host_consts()
    scale = 1.0 / math.sqrt(DK)
    wqk = np.concatenate([(Wq * scale).astype(bf), Wk.astype(bf)], axis=1)
    csts = np.concatenate([band01T, m168T, m24T, ident], axis=1)
    base = {
        "Wqk_s": np.ascontiguousarray(wqk),
        "Wo": np.ascontiguousarray((Wo / 3.0).astype(bf)),
        "csts": np.ascontiguousarray(csts),
    }
    if has_bq:
        base["bq_s"] = np.ascontiguousarray(bq * scale, dtype=np.float32).reshape(DK, 1)
    if has_bk:
        base["bk_c"] = np.ascontiguousarray(bk, dtype=np.float32).reshape(DK, 1)
    if has_bo:
        base["ones_row"] = np.ones((1, 128), dtype=np.float32).astype(bf)
        base["bo_row"] = (bo / 3.0).astype(bf).reshape(1, D)
    if has_gamma:
        base["gamma_bc"] = np.broadcast_to(
            np.asarray(gamma, dtype=np.float32), (128, D)
        ).copy()
    if has_beta:
        base["beta_bc"] = np.broadcast_to(
            np.asarray(beta, dtype=np.float32), (128, D)
        ).copy()
    xb = np.ascontiguousarray(x).astype(bf)
    return [{**base, "x": xb[core]} for core in range(B)]


def kernel(x, Wq, bq, Wk, bk, Wo, bo, gamma, beta):
    from concourse.bass_utils import run_bass_kernel_spmd

    x = np.asarray(x, dtype=np.float32)
    Wq = np.asarray(Wq, dtype=np.float32)
    bq = np.asarray(bq, dtype=np.float32)
    Wk = np.asarray(Wk, dtype=np.float32)
    bk = np.asarray(bk, dtype=np.float32)
    Wo = np.asarray(Wo, dtype=np.float32)
    bo = np.asarray(bo, dtype=np.float32)
    gamma = np.asarray(gamma, dtype=np.float32)
    beta = np.asarray(beta, dtype=np.float32)

    flags = (
        bool(np.any(bq != 0.0)),
        bool(np.any(bk != 0.0)),
        bool(np.any(bo != 0.0)),
        bool(np.any(gamma != 1.0)),
        bool(np.any(beta != 0.0)),
    )
    nc = _get_built(flags)
    in_maps = _make_in_maps(x, Wq, bq, Wk, bk, Wo, bo, gamma, beta, flags)
    res = run_bass_kernel_spmd(nc, in_maps, list(range(B)))
    return np.stack(
        [np.asarray(res.results[c]["out"], dtype=np.float32) for c in range(B)], axis=0
    )
